# revision 40
# baseline (speedup 1.0000x reference)
"""Trainium2 Bass kernel for a CQT (constant-Q transform) nn.Module.

Reference computation (per batch sample b, channel c):
    out[b, c, k, f, 0] = sum_t x[b, c, f*HOP + t] * w_re[k, t]
    out[b, c, k, f, 1] = sum_t x[b, c, f*HOP + t] * w_im[k, t]
where w_re/w_im are Hann-windowed complex exponentials with per-bin ragged
lengths (longest 11340 samples), HOP=512, 84 bins, 409 frames.

Strategy: data-parallel over the batch (1 sample per NeuronCore, 8 cores).
Per core the correlation is factored through a two-level block basis with
pair-packed contractions, cutting tensor-engine moving rows ~8x vs the
direct banded matmul (9316 rows vs 75440):

  1. Stage 1: each 512-sample hop block of x is projected onto a 128-dim
     orthonormal basis B0 = [Bp | Br]: Bp (64) spans the 512-pieces of the
     level-2 basis (exact to 8e-14), Br (64) the residual of the short
     bins' windows.  4 matmuls x 436 moving rows per channel.
  2. Stage 2: coefficients of 4 adjacent blocks are combined into
     2048-block coefficients in basis B2 (SVD of the 2048-aligned segments
     of bins 0..41; 64 components, tail 3e-10).  Because the combiners
     C_j live entirely in the Bp half, adjacent j are packed into the two
     partition halves of a shifted-stacked Y0 copy: 2 matmuls x 432.
  3. Final: bins 42..83 (windows <= 1002 samples, 1-2 hop blocks)
     correlate against Y0 at shifts 0,1 (2 matmuls x 410); bins 0..41
     correlate against Y2 at 6 block shifts, pair-packed via a
     shifted-stacked Y2 copy (3 matmuls x 410), ragged row prefixes
     accumulating in one PSUM bank.

The basis + coefficient payload is ~100 KB vs 7.9 MB of raw windows.
Everything runs in fp16 (measured end-to-end relative error vs the fp32
reference ~5e-4); output is written fp16 and widened on the host.

Scheduling notes (cost-model driven):
  - DMA completion in the cost model is seq-slot (500 ns) + ~1.3 us DGE
    pipeline + transfer, so inputs are spread over all three DMA queues
    (sync/SP, scalar-queue/Act, gpsimd/SWDGE) with the first-needed tiles
    first; wb0 rides as a 128-column header on ch0's x tiles.
  - No activation-engine compute is emitted (a LoadActFuncSet would stall
    the Act queue's DMAs by 1.3 us).
  - PSUM->SBUF casts are split between DVE and the Pool ALU.
  - The final stage is split into column halves so the first half's
    writeback overlaps the second half's matmuls; output rides fp16.
"""

import math
import os as _os
from contextlib import ExitStack

import numpy as np

import concourse.bass as bass
import concourse.mybir as mybir
import concourse.tile as tile
from concourse import bacc
from concourse.bass_utils import run_bass_kernel_spmd

# ---- problem constants (hardcoded CQT spec) ----
SR = 22050
N_BINS = 84
BPO = 12
FMIN = 32.7
HOP = 512
B, C, T = 8, 2, 220500
N_CORES = 8

LMAX = 11340           # longest window
F = 409                # frames: 1 + (T - LMAX)//HOP
FP = 410               # even moving-dim padding for the final stage
FH = 206               # final-stage column split: [0:FH) and [FH:FP)
NB0 = 436              # 512-sample x blocks incl. pad (431 real + shift room)
NB2 = 432              # 2048-block coefficient positions (F + 4*5 + pad)
HDR = 128              # wb0 header columns on ch0 x tiles
KL2 = 42               # bins 0..KL2-1 via 2048-blocks, rest via 512-blocks
Q2 = 64                # level-2 basis size (fits one partition half)
NROWS = 2 * N_BINS
RA = NROWS - 2 * KL2   # 84 rows (bins 42..83) through the level-0 path
RB = 2 * KL2           # 84 rows (bins 0..41) through the level-2 path

TL_DT = mybir.dt.float16

# copy-engine assignment: y0f(0), y0f(1), y2s(0), y2s(1), oa0, oa1, ob0, ob1
# ('v' = DVE, 's' = Activation); tuned by cost-model sweep
K_CFG = _os.environ.get("K_CFG", "vvssvsvs")

_PREP = None
_NC = None
LAST_RESULTS = None


def _params():
    """Host-side constants: bases + projection coefficients (float64 SVD)."""
    global _PREP
    if _PREP is not None:
        return _PREP

    Qf = 1.0 / (2.0 ** (1.0 / BPO) - 1.0)
    freqs = FMIN * 2.0 ** (np.arange(N_BINS, dtype=np.float64) / BPO)
    lengths = np.round(Qf * SR / freqs).astype(np.int64)
    assert int(lengths.max()) == LMAX

    t = np.arange(LMAX, dtype=np.float64)
    L = lengths.astype(np.float64)[:, None]
    mask = (t[None, :] < L).astype(np.float64)
    win = 0.5 * (1.0 - np.cos(2.0 * math.pi * t[None, :] / L)) * mask
    phase = (2.0 * math.pi / SR) * freqs[:, None] * t[None, :]
    W = np.zeros((NROWS, 6 * 2048), dtype=np.float64)
    W[0::2, :LMAX] = win * np.cos(phase)
    W[1::2, :LMAX] = -win * np.sin(phase)

    def seg_matrix(bins, blk):
        out = []
        for k in bins:
            for u in range(math.ceil(int(lengths[k]) / blk)):
                out.append(W[2 * k, blk * u:blk * (u + 1)])
                out.append(W[2 * k + 1, blk * u:blk * (u + 1)])
        return np.array(out)

    # B2: 64-dim basis of the 2048-aligned segments of bins 0..KL2-1
    _, _, V2 = np.linalg.svd(seg_matrix(range(KL2), 2048),
                             full_matrices=False)
    B2 = V2[:Q2]                                    # (64, 2048)
    # Bp: 64-dim basis of B2's 512-pieces (numerically exact)
    pieces = np.concatenate([B2[:, 512 * j:512 * (j + 1)] for j in range(4)],
                            axis=0)
    _, _, Vp = np.linalg.svd(pieces, full_matrices=False)
    Bp = Vp[:64]
    # Br: 64-dim basis of the short bins' segments, residual to Bp
    M0t = seg_matrix(range(KL2, N_BINS), 512)
    resid = M0t - (M0t @ Bp.T) @ Bp
    _, _, Vr = np.linalg.svd(resid, full_matrices=False)
    B0 = np.concatenate([Bp, Vr[:64]], axis=0)      # (128, 512) orthonormal

    # stage-2 combiners, zero-padded to full 128 contraction rows so s2 can
    # read the same full Y0 copy the final stage uses
    Cj = [np.concatenate([Bp @ B2[:, 512 * j:512 * (j + 1)].T,
                          np.zeros((64, Q2))], axis=0)
          for j in range(4)]                        # (128, 64) each
    # final-stage coefficients
    A0 = [B0 @ W[2 * KL2:, 512 * u:512 * (u + 1)].T for u in range(2)]
    rows0 = [RA, 2 * int((np.ceil(lengths[KL2:] / 512.0) > 1).sum())]  # 84,24
    nb2 = np.ceil(lengths[:KL2] / 2048.0).astype(np.int64)
    U2 = int(nb2.max())                             # 6
    rows_u2 = [2 * int((nb2 > u).sum()) for u in range(U2)]
    A2 = [B2 @ W[:rows_u2[u], 2048 * u:2048 * (u + 1)].T for u in range(U2)]
    # f2 pair blocks: [A2_{2t} (q<64); A2_{2t+1} zero-padded (64+q)]
    A2p = []
    for tp in range(U2 // 2):
        r0, r1 = rows_u2[2 * tp], rows_u2[2 * tp + 1]
        blk = np.zeros((128, r0))
        blk[:64] = A2[2 * tp]
        blk[64:, :r1] = A2[2 * tp + 1]
        A2p.append(blk)

    # wb0[r, 128*rc + p] = B0[p, 128*rc + r]  (stationary for stage-1 chunk rc)
    wb0 = np.zeros((128, 512), dtype=np.float16)
    for rc in range(4):
        wb0[:, 128 * rc:128 * (rc + 1)] = B0[:, 128 * rc:128 * (rc + 1)].T
    # wrest = [C_0..C_3 | A0_0 | A0_1 | A2 pairs]
    blocks = Cj + [A0[0], A0[1]] + A2p
    offs = np.cumsum([0] + [b.shape[1] for b in blocks])
    wrest = np.zeros((128, int(offs[-1])), dtype=np.float16)
    for b_, o in zip(blocks, offs[:-1]):
        wrest[:, int(o):int(o) + b_.shape[1]] = b_

    _PREP = dict(wb0=wb0, wrest=wrest, offs=[int(o) for o in offs],
                 rows0=rows0, rows_u2=rows_u2, U2=U2)
    return _PREP


def _build_nc():
    p = _params()
    offs, rows0, rows_u2 = p["offs"], p["rows0"], p["rows_u2"]
    WREST = offs[-1]

    nc = bacc.Bacc(None, target_bir_lowering=False)
    xh_d = nc.dram_tensor("xh", (4, 128, HDR + NB0), TL_DT,
                          kind="ExternalInput")
    x1_d = nc.dram_tensor("x1", (4, 128, NB0), TL_DT, kind="ExternalInput")
    wrest_d = nc.dram_tensor("wrest", (128, WREST), TL_DT,
                             kind="ExternalInput")
    out_d = nc.dram_tensor("out", (C, NROWS, F), TL_DT, kind="ExternalOutput")

    with ExitStack() as ctx:
        tc = ctx.enter_context(tile.TileContext(nc))
        xp = ctx.enter_context(tc.tile_pool(name="xp", bufs=1))
        wp = ctx.enter_context(tc.tile_pool(name="wp", bufs=1))
        yp = ctx.enter_context(tc.tile_pool(name="yp", bufs=1))
        op = ctx.enter_context(tc.tile_pool(name="op", bufs=1))
        pp = ctx.enter_context(tc.tile_pool(name="pp", bufs=1, space="PSUM"))

        # PSUM: 4 banks per channel (Y0, Y2, outA, outB) = all 8 banks
        y0_ps = {ch: pp.tile([128, 512], mybir.dt.float32, name=f"y0p_{ch}",
                             tag=f"y0p_{ch}") for ch in range(C)}
        y2_ps = {ch: pp.tile([128, 512], mybir.dt.float32, name=f"y2p_{ch}",
                             tag=f"y2p_{ch}") for ch in range(C)}
        oa_ps = {ch: pp.tile([128, 512], mybir.dt.float32, name=f"oap_{ch}",
                             tag=f"oap_{ch}") for ch in range(C)}
        ob_ps = {ch: pp.tile([128, 512], mybir.dt.float32, name=f"obp_{ch}",
                             tag=f"obp_{ch}") for ch in range(C)}

        # --- SBUF tiles ---
        xh_sb = {rc: xp.tile([128, HDR + NB0], TL_DT, name=f"xh_{rc}",
                             tag=f"xh_{rc}") for rc in range(4)}
        x1_sb = {rc: xp.tile([128, NB0], TL_DT, name=f"x1_{rc}",
                             tag=f"x1_{rc}") for rc in range(4)}
        wrest_sb = wp.tile([128, WREST], TL_DT, name="wrest_sb",
                           tag="wrest_sb")
        y0f_sb = {ch: yp.tile([128, NB0], TL_DT, name=f"y0f_{ch}",
                              tag=f"y0f_{ch}") for ch in range(C)}
        y2s_sb = {ch: yp.tile([128, NB2], TL_DT, name=f"y2s_{ch}",
                              tag=f"y2s_{ch}") for ch in range(C)}

        # --- input DMA plan (three queues, first-needed tiles first).
        # The scalar engine does PSUM readback copies, so its queue leads
        # with a 1.3us LoadActFuncSet; only the latest-needed x tile rides
        # behind it.  GPSIMD compute cannot touch PSUM (BIR rule), so the
        # Pool queue is input DMAs only.
        nc.sync.dma_start(xh_sb[0][:], xh_d[0])
        nc.sync.dma_start(xh_sb[2][:], xh_d[2])
        nc.sync.dma_start(x1_sb[0][:], x1_d[0])
        nc.sync.dma_start(x1_sb[3][:], x1_d[3])
        nc.gpsimd.dma_start(xh_sb[1][:], xh_d[1])
        nc.gpsimd.dma_start(xh_sb[3][:], xh_d[3])
        nc.gpsimd.dma_start(x1_sb[1][:], x1_d[1])
        nc.gpsimd.dma_start(wrest_sb[:], wrest_d[:])
        nc.scalar.dma_start(x1_sb[2][:], x1_d[2])

        # --- PE stream ---
        def s1(ch):
            for rc in range(4):
                mov = (xh_sb[rc][:, HDR:HDR + NB0] if ch == 0
                       else x1_sb[rc][:, 0:NB0])
                nc.tensor.matmul(y0_ps[ch][0:128, 0:NB0],
                                 xh_sb[rc][:, 0:HDR], mov,
                                 start=(rc == 0), stop=(rc == 3),
                                 skip_group_check=True)

        SH = 216  # stage-2 column split

        def s2h(ch, c0, c1):
            for j in range(4):
                nc.tensor.matmul(y2_ps[ch][0:Q2, c0:c1],
                                 wrest_sb[:, 64 * j:64 * (j + 1)],
                                 y0f_sb[ch][:, c0 + j:c1 + j],
                                 start=(j == 0), stop=(j == 3),
                                 skip_group_check=True)

        def f0(ch):
            for u in range(2):
                nc.tensor.matmul(oa_ps[ch][0:rows0[u], 0:FP],
                                 wrest_sb[:, offs[4 + u]:offs[4 + u]
                                          + rows0[u]],
                                 y0f_sb[ch][:, u:u + FP],
                                 start=(u == 0), stop=(u == 1),
                                 skip_group_check=True)

        def f2(ch, ps, c0, c1):
            for tp in range(3):
                m = rows_u2[2 * tp]
                off = offs[6 + tp]
                nc.tensor.matmul(ps[0:m, 0:c1 - c0],
                                 wrest_sb[:, off:off + m],
                                 y2s_sb[ch][:, 8 * tp + c0:8 * tp + c1],
                                 start=(tp == 0), stop=(tp == 2),
                                 skip_group_check=True)

        # --- PSUM -> SBUF casts, balanced across the two PSUM-capable
        # engines (DVE and Activation; GPSIMD may not touch PSUM) ---
        def _cp(i):
            return (nc.vector.tensor_copy if K_CFG[i] == "v"
                    else nc.scalar.copy)

        def copy_y0(ch):
            _cp(ch)(y0f_sb[ch][:, 0:SH + 4], y0_ps[ch][:, 0:SH + 4])
            _cp(ch)(y0f_sb[ch][:, SH + 4:NB0], y0_ps[ch][:, SH + 4:NB0])

        def copy_y2(ch):
            # stacked-shift copies: rows 64+q hold Y2[q, m+4] for the f2
            # pair-packing
            eng = _cp(2 + ch)
            eng(y2s_sb[ch][0:64, 0:NB2], y2_ps[ch][0:64, 0:NB2])
            eng(y2s_sb[ch][64:128, 0:NB2 - 4], y2_ps[ch][0:64, 4:NB2])

        FB = 272  # f2(1) column split: [0:FB) -> ob_ps[1], rest -> y2_ps[0]

        def out_a(ch, queue):
            o = op.tile([RA, F], TL_DT, name=f"oa{ch}", tag=f"oa{ch}")
            _cp(4 + ch)(o[:], oa_ps[ch][0:RA, 0:F])
            queue.dma_start(out_d[ch, RB:NROWS, :], o[:])

        def out_b(ch, queue):
            o = op.tile([RB, F], TL_DT, name=f"ob{ch}", tag=f"ob{ch}")
            _cp(6 + ch)(o[:], ob_ps[ch][0:RB, 0:F])
            queue.dma_start(out_d[ch, 0:RB, :], o[:])

        s1(0)
        copy_y0(0)
        s1(1)
        copy_y0(1)
        s2h(0, 0, SH)
        s2h(0, SH, NB2)
        # pin ch0's stacked copies ahead of ch1's on the Act queue; the
        # scheduler otherwise interleaves channels and delays f2(0)
        with tc.high_priority():
            copy_y2(0)
        s2h(1, 0, SH)
        s2h(1, SH, NB2)
        copy_y2(1)
        f0(0)
        out_a(0, nc.sync)
        f0(1)
        out_a(1, nc.gpsimd)
        f2(0, ob_ps[0], 0, FP)
        out_b(0, nc.sync)
        f2(1, ob_ps[1], 0, FP)
        out_b(1, nc.scalar)
    nc.finalize()
    return nc


def get_nc():
    global _NC
    if _NC is None:
        _NC = _build_nc()
    return _NC


def _pack_x(xb):
    """(C, T) -> (C, 4, 128, NB0) fp16 with
    xt[ch, rc, r, m] = x[ch, 512*m + 128*rc + r]."""
    xpad = np.zeros((C, NB0 * 512), dtype=np.float32)
    xpad[:, :T] = xb
    return np.ascontiguousarray(
        xpad.reshape(C, NB0, 4, 128).transpose(0, 2, 3, 1)).astype(np.float16)


def make_inputs(xb):
    """Per-core input map for one batch sample xb of shape (C, T)."""
    p = _params()
    xt = _pack_x(xb)
    xh = np.zeros((4, 128, HDR + NB0), dtype=np.float16)
    for rc in range(4):
        xh[rc, :, :HDR] = p["wb0"][:, 128 * rc:128 * (rc + 1)]
        xh[rc, :, HDR:] = xt[0, rc]
    return {"xh": xh, "x1": xt[1], "wrest": p["wrest"]}


def kernel(x):
    global LAST_RESULTS
    x = np.asarray(x, dtype=np.float32)
    assert x.shape == (B, C, T)
    in_maps = [make_inputs(x[b]) for b in range(B)]
    nc = get_nc()
    res = run_bass_kernel_spmd(nc, in_maps, core_ids=list(range(N_CORES)))
    LAST_RESULTS = res
    out = np.empty((B, C, N_BINS, F, 2), dtype=np.float32)
    for b in range(B):
        raw = np.asarray(res.results[b]["out"]).astype(np.float32)
        out[b] = raw.reshape(C, N_BINS, 2, F).transpose(0, 1, 3, 2)
    return out
